# revision 19
# baseline (speedup 1.0000x reference)
"""Multi-head attention (B=2, L=2048, DIM=2048, H=16) on 8 TRN2 NeuronCores.

Sharding: data-parallel over batch (2) x tensor-parallel over head groups (4).
Core c handles batch c//4, heads [4*(c%4), 4*(c%4)+4): it receives the full
query/value tensors for its batch plus the 512-row slices of Wq/Wk/Wv for its
heads, and produces the [2048, 512] slice of the output.

Math per core (head h, dh=128):
  q = xq_masked @ WqT  (feature-major Q_T[j, l]);  k, v likewise from xv
  S_T[k, q] = K_T . Q_T  (float32r matmuls, full PE rate)
  E_T = exp(S_T / sqrt(128))  (bf16; query mask pre-folded into xq rows)
  attn[q, 0:128] / den[q, 128] from one bf16 matmul with a ones-column
  appended to V; out = attn * (1/den) per partition.
"""

import sys

for p in ("/opt/trn_rl_repo", "/opt/pypackages"):
    if p not in sys.path:
        sys.path.insert(0, p)

import numpy as np

import concourse.bacc as bacc
import concourse.bass as bass
import concourse.mybir as mybir
import concourse.tile as tile
from concourse import masks
from concourse.bass_utils import run_bass_kernel_spmd

N_CORES = 8
B, L, DIM, H = 2, 2048, 2048, 16
JB = DIM // 4          # 512 output features per core (4 heads)
DH = 128               # head dim
NH = 4                 # heads per core
NDC = DIM // 128       # 16 contraction chunks
LB = 256               # l-block for QKV staging (fp32r needs N>=256)
NLB = L // LB          # 8
NKT = L // 128         # 16 k tiles
SCALE = 1.0 / np.sqrt(DH)

F32 = mybir.dt.float32
F32R = mybir.dt.float32r
BF16 = mybir.dt.bfloat16


def build_nc():
    nc = bacc.Bacc("TRN2", target_bir_lowering=False, debug=False,
                   num_devices=N_CORES)
    xq = nc.dram_tensor("xq", [L, DIM], F32, kind="ExternalInput").ap()
    xv = nc.dram_tensor("xv", [L, DIM], F32, kind="ExternalInput").ap()
    msk = nc.dram_tensor("msk", [L, 1], F32, kind="ExternalInput").ap()
    wq = nc.dram_tensor("wq", [JB, DIM], F32, kind="ExternalInput").ap()
    wk = nc.dram_tensor("wk", [JB, DIM], F32, kind="ExternalInput").ap()
    wv = nc.dram_tensor("wv", [JB, DIM], F32, kind="ExternalInput").ap()
    out = nc.dram_tensor("out", [L, JB], F32, kind="ExternalOutput").ap()

    with tile.TileContext(nc) as tc:
        build_kernel(nc, tc, xq, xv, msk, wq, wk, wv, out)
    nc.compile()
    return nc


def transpose_w(nc, tc, ctx, w_dram, name):
    """DMA W [512, 2048]; produce per-dc tiles wt[dc] [128 d, 512 j].
    Transposes are 2-packed per PSUM bank -> copies of [128, 256]."""
    wt_pool = ctx.enter_context(tc.tile_pool(name=f"{name}t", bufs=1))
    wt = [wt_pool.tile([128, JB], F32R, tag=f"{name}{dc}", name=f"{name}T{dc}")
          for dc in range(NDC)]
    with tc.tile_pool(name=f"{name}_stage", bufs=2) as stage, \
         tc.tile_pool(name=f"{name}_ps", bufs=2, space="PSUM") as ps:
        for jp in range(JB // 256):          # pairs of j row-tiles
            rows = []
            for i in range(2):
                w_rows = stage.tile([128, DIM], F32, tag="wrow",
                                    name=f"{name}_rows{jp}_{i}")
                nc.sync.dma_start(
                    w_rows[:],
                    w_dram[(jp * 2 + i) * 128:(jp * 2 + i + 1) * 128, :])
                rows.append(w_rows)
            for dc in range(NDC):
                pt = ps.tile([128, 256], F32, tag="wps", name=f"{name}_ps")
                for i in range(2):
                    nc.tensor.transpose(
                        pt[:, i * 128:(i + 1) * 128],
                        rows[i][:, dc * 128:(dc + 1) * 128], tc.ident[:])
                nc.vector.tensor_copy(
                    wt[dc][:, jp * 256:(jp + 1) * 256], pt[:])
    return wt


def project(nc, tc, ctx, x_dram, wts, mask_dram=None):
    """QKV projection. wts: list of (wt_tiles, kind, dst): kind 'T' makes
    feature-major [128 j, L] tiles in dst (4 tiles [128, L]); kind 'V'
    makes seq-major bf16+ones V tiles (dst: NKT tiles [128, NH*129])."""
    nlt = LB // 128
    with tc.tile_pool(name="x_stage", bufs=2) as xstage, \
         tc.tile_pool(name="xt_stage", bufs=1) as xtstage, \
         tc.tile_pool(name="m_stage", bufs=2) as mstage, \
         tc.tile_pool(name="tp_ps", bufs=2, space="PSUM") as tps, \
         tc.tile_pool(name="acc_ps", bufs=1, space="PSUM") as aps, \
         tc.tile_pool(name="accv_ps", bufs=1, space="PSUM") as vps:
        for lb in range(NLB):
            xt = [xtstage.tile([128, LB], F32R, tag=f"xt{dc}",
                               name=f"xt{dc}") for dc in range(NDC)]
            rows = []
            for lt in range(nlt):
                l0 = lb * LB + lt * 128
                xrow = xstage.tile([128, DIM], F32, tag=f"xrow{lt}",
                                   name=f"xrow{lt}")
                nc.sync.dma_start(xrow[:], x_dram[l0:l0 + 128, :])
                if mask_dram is not None:
                    mt = mstage.tile([128, 1], F32, tag="mt", name="mt")
                    nc.sync.dma_start(mt[:], mask_dram[l0:l0 + 128, :])
                    nc.vector.tensor_scalar_mul(xrow[:], xrow[:], mt[:])
                rows.append(xrow)
            for dc in range(NDC):
                pt = tps.tile([128, LB], F32, tag="tp", name="tp")
                for lt in range(nlt):
                    nc.tensor.transpose(
                        pt[:, lt * 128:(lt + 1) * 128],
                        rows[lt][:, dc * 128:(dc + 1) * 128], tc.ident[:])
                nc.vector.tensor_copy(xt[dc][:], pt[:])
            # accumulation, dc-outer so matmuls chase the transposes
            accs = {}
            for wi, (wt, kind, dst) in enumerate(wts):
                if kind == "T":
                    accs[wi] = [aps.tile([128, LB], F32, tag=f"accT{jt}",
                                         name=f"accT{jt}")
                                for jt in range(JB // 128)]
                else:
                    accs[wi] = [vps.tile([128, JB], F32, tag=f"accV{lt}",
                                         name=f"accV{lt}")
                                for lt in range(nlt)]
            for dc in range(NDC):
                for wi, (wt, kind, dst) in enumerate(wts):
                    if kind == "T":
                        for jt in range(JB // 128):
                            nc.tensor.matmul(
                                accs[wi][jt][:],
                                wt[dc][:, jt * 128:(jt + 1) * 128],
                                xt[dc][:],
                                start=(dc == 0), stop=(dc == NDC - 1))
                    else:
                        for lt in range(nlt):
                            nc.tensor.matmul(
                                accs[wi][lt][:],
                                xt[dc][:, lt * 128:(lt + 1) * 128],
                                wt[dc][:],
                                start=(dc == 0), stop=(dc == NDC - 1))
            for wi, (wt, kind, dst) in enumerate(wts):
                if kind == "T":
                    for jt in range(JB // 128):
                        nc.vector.tensor_copy(
                            dst[jt][:, lb * LB:(lb + 1) * LB],
                            accs[wi][jt][:])
                else:
                    for lt in range(nlt):
                        kt = lb * nlt + lt
                        for h in range(NH):
                            nc.vector.tensor_copy(
                                dst[kt][:, h * 129: h * 129 + 128],
                                accs[wi][lt][:, h * 128:(h + 1) * 128])


def build_kernel(nc, tc, xq, xv, msk, wq, wk, wv, out):
    import contextlib
    octx = contextlib.ExitStack()
    with octx:
        const_pool = octx.enter_context(tc.tile_pool(name="const", bufs=1))
        tc.ident = const_pool.tile([128, 128], F32)
        masks.make_identity(nc, tc.ident[:])

        # persistent products
        qt_pool = octx.enter_context(tc.tile_pool(name="qt", bufs=1))
        kt_pool = octx.enter_context(tc.tile_pool(name="kt", bufs=1))
        v_pool = octx.enter_context(tc.tile_pool(name="v", bufs=1))
        Q_T = [qt_pool.tile([128, L], F32R, tag=f"q{h}", name=f"qT{h}")
               for h in range(NH)]
        K_T = [kt_pool.tile([128, L], F32R, tag=f"k{h}", name=f"kT{h}")
               for h in range(NH)]
        V = [v_pool.tile([128, NH * 129], BF16, tag=f"v{t}", name=f"vS{t}")
             for t in range(NKT)]
        for t in range(NKT):
            nc.vector.memset(V[t][:], 1.0)

        # ---- phase Q ----
        with contextlib.ExitStack() as qctx:
            wqt = transpose_w(nc, tc, qctx, wq, "wq")
            project(nc, tc, qctx, xq, [(wqt, "T", Q_T)], mask_dram=msk)

        # ---- phase K/V ----
        with contextlib.ExitStack() as kvctx:
            wkt = transpose_w(nc, tc, kvctx, wk, "wk")
            wvt = transpose_w(nc, tc, kvctx, wv, "wv")
            project(nc, tc, kvctx, xv, [(wkt, "T", K_T), (wvt, "V", V)])

        # ---- attention ----
        with tc.tile_pool(name="et", bufs=2) as et_pool, \
             tc.tile_pool(name="s_ps", bufs=3, space="PSUM") as sps, \
             tc.tile_pool(name="a_ps", bufs=3, space="PSUM") as aps, \
             tc.tile_pool(name="o_sb", bufs=8) as osb, \
             tc.tile_pool(name="r_sb", bufs=4) as rsb:
            for qb in range(L // 512):
                ots = [osb.tile([128, JB], F32, tag=f"ot{qs}",
                                name=f"ot{qs}") for qs in range(4)]
                for h in range(NH):
                    et = [et_pool.tile([128, 512], BF16, tag=f"et{kc}",
                                       name=f"et{kc}") for kc in range(NKT)]
                    for kc in range(NKT):
                        s = sps.tile([128, 512], F32, tag="s", name="s")
                        nc.tensor.matmul(
                            s[:],
                            K_T[h][:, kc * 128:(kc + 1) * 128],
                            Q_T[h][:, qb * 512:(qb + 1) * 512],
                            start=True, stop=True)
                        nc.scalar.activation(
                            et[kc][:], s[:],
                            mybir.ActivationFunctionType.Exp,
                            scale=float(SCALE))
                    for qs in range(4):
                        a = aps.tile([128, 129], F32, tag="a", name="a")
                        for kc in range(NKT):
                            nc.tensor.matmul(
                                a[:],
                                et[kc][:, qs * 128:(qs + 1) * 128],
                                V[kc][:, h * 129:(h + 1) * 129],
                                start=(kc == 0), stop=(kc == NKT - 1))
                        rec = rsb.tile([128, 1], F32, tag="rec", name="rec")
                        nc.vector.reciprocal(rec[:], a[:, 128:129])
                        nc.vector.tensor_scalar_mul(
                            ots[qs][:, h * 128:(h + 1) * 128],
                            a[:, 0:128], rec[:])
                for qs in range(4):
                    q0 = qb * 512 + qs * 128
                    nc.sync.dma_start(out[q0:q0 + 128, :], ots[qs][:])


_NC_CACHE = None


def _get_nc():
    global _NC_CACHE
    if _NC_CACHE is None:
        _NC_CACHE = build_nc()
    return _NC_CACHE


def make_in_maps(query_tensor, value_tensor, attention_mask, Wq, Wk, Wv):
    in_maps = []
    for c in range(N_CORES):
        b, g = divmod(c, 4)
        j0 = g * JB
        in_maps.append({
            "xq": np.ascontiguousarray(query_tensor[b], dtype=np.float32),
            "xv": np.ascontiguousarray(value_tensor[b], dtype=np.float32),
            "msk": np.ascontiguousarray(
                attention_mask[b].reshape(L, 1), dtype=np.float32),
            "wq": np.ascontiguousarray(Wq[j0:j0 + JB], dtype=np.float32),
            "wk": np.ascontiguousarray(Wk[j0:j0 + JB], dtype=np.float32),
            "wv": np.ascontiguousarray(Wv[j0:j0 + JB], dtype=np.float32),
        })
    return in_maps


def assemble(results):
    out = np.empty((B, L, DIM), dtype=np.float32)
    for c in range(N_CORES):
        b, g = divmod(c, 4)
        out[b, :, g * JB:(g + 1) * JB] = results[c]["out"]
    return out


def kernel(query_tensor, value_tensor, attention_mask, Wq, Wk, Wv):
    nc = _get_nc()
    in_maps = make_in_maps(np.asarray(query_tensor), np.asarray(value_tensor),
                           np.asarray(attention_mask), np.asarray(Wq),
                           np.asarray(Wk), np.asarray(Wv))
    res = run_bass_kernel_spmd(nc, in_maps, core_ids=list(range(N_CORES)))
    return assemble(res.results)


# revision 20
# speedup vs baseline: 1.6496x; 1.6496x over previous
"""Multi-head attention (B=2, L=2048, DIM=2048, H=16) on 8 TRN2 NeuronCores.

Sharding: data-parallel over batch (2) x tensor-parallel over head groups (4).
Core c handles batch c//4, heads [4*(c%4), 4*(c%4)+4): it receives the full
query/value tensors for its batch plus the 512-row slices of Wq/Wk/Wv for its
heads, and produces the [2048, 512] slice of the output.

Math per core (head h, dh=128):
  q = xq_masked @ WqT  (feature-major Q_T[j, l]);  k, v likewise from xv
  S_T[k, q] = K_T . Q_T  (float32r matmuls, full PE rate)
  E_T = exp(S_T / sqrt(128))  (bf16; query mask pre-folded into xq rows)
  attn[q, 0:128] / den[q, 128] from one bf16 matmul with a ones-column
  appended to V; out = attn * (1/den) per partition.
"""

import sys

for p in ("/opt/trn_rl_repo", "/opt/pypackages"):
    if p not in sys.path:
        sys.path.insert(0, p)

import numpy as np

import concourse.bacc as bacc
import concourse.bass as bass
import concourse.mybir as mybir
import concourse.tile as tile
from concourse import masks
from concourse.bass_utils import run_bass_kernel_spmd

N_CORES = 8
B, L, DIM, H = 2, 2048, 2048, 16
JB = DIM // 4          # 512 output features per core (4 heads)
DH = 128               # head dim
NH = 4                 # heads per core
NDC = DIM // 128       # 16 contraction chunks
LB = 256               # l-block for QKV staging (fp32r needs N>=256)
NLB = L // LB          # 8
NKT = L // 128         # 16 k tiles
SCALE = 1.0 / np.sqrt(DH)

F32 = mybir.dt.float32
F32R = mybir.dt.float32r
BF16 = mybir.dt.bfloat16


def act_copy(nc, out_ap, in_ap):
    nc.scalar.activation(out_ap, in_ap, mybir.ActivationFunctionType.Copy)


def build_nc():
    nc = bacc.Bacc("TRN2", target_bir_lowering=False, debug=False,
                   num_devices=N_CORES)
    xq = nc.dram_tensor("xq", [L, DIM], F32, kind="ExternalInput").ap()
    xv = nc.dram_tensor("xv", [L, DIM], F32, kind="ExternalInput").ap()
    msk = nc.dram_tensor("msk", [L, 1], F32, kind="ExternalInput").ap()
    wq = nc.dram_tensor("wq", [JB, DIM], F32, kind="ExternalInput").ap()
    wk = nc.dram_tensor("wk", [JB, DIM], F32, kind="ExternalInput").ap()
    wv = nc.dram_tensor("wv", [JB, DIM], F32, kind="ExternalInput").ap()
    out = nc.dram_tensor("out", [L, JB], F32, kind="ExternalOutput").ap()

    with tile.TileContext(nc) as tc:
        build_kernel(nc, tc, xq, xv, msk, wq, wk, wv, out)
    nc.compile()
    return nc


def transpose_w(nc, tc, ctx, w_dram, name):
    """DMA W [512, 2048]; produce per-dc tiles wt[dc] [128 d, 512 j].
    Transposes are 2-packed per PSUM bank -> copies of [128, 256]."""
    wt_pool = ctx.enter_context(tc.tile_pool(name=f"{name}t", bufs=1))
    wt = [wt_pool.tile([128, JB], F32R, tag=f"{name}{dc}", name=f"{name}T{dc}")
          for dc in range(NDC)]
    with tc.tile_pool(name=f"{name}_stage", bufs=2) as stage, \
         tc.tile_pool(name=f"{name}_ps", bufs=2, space="PSUM") as ps:
        for jp in range(JB // 256):          # pairs of j row-tiles
            rows = []
            for i in range(2):
                w_rows = stage.tile([128, DIM], F32, tag="wrow",
                                    name=f"{name}_rows{jp}_{i}")
                nc.sync.dma_start(
                    w_rows[:],
                    w_dram[(jp * 2 + i) * 128:(jp * 2 + i + 1) * 128, :])
                rows.append(w_rows)
            for dc in range(NDC):
                pt = ps.tile([128, 256], F32, tag="wps", name=f"{name}_ps")
                for i in range(2):
                    nc.tensor.transpose(
                        pt[:, i * 128:(i + 1) * 128],
                        rows[i][:, dc * 128:(dc + 1) * 128], tc.ident[:])
                nc.vector.tensor_copy(
                    wt[dc][:, jp * 256:(jp + 1) * 256], pt[:])
    return wt


def project(nc, tc, ctx, x_dram, wts, mask_dram=None):
    """QKV projection. wts: list of (wt_tiles, kind, dst): kind 'T' makes
    feature-major [128 j, L] tiles in dst (4 tiles [128, L]); kind 'V'
    makes seq-major bf16+ones V tiles (dst: NKT tiles [128, NH*129])."""
    nlt = LB // 128
    with tc.tile_pool(name="x_stage", bufs=2) as xstage, \
         tc.tile_pool(name="xt_stage", bufs=1) as xtstage, \
         tc.tile_pool(name="m_stage", bufs=2) as mstage, \
         tc.tile_pool(name="tp_ps", bufs=2, space="PSUM") as tps, \
         tc.tile_pool(name="acc_ps", bufs=2, space="PSUM") as aps, \
         tc.tile_pool(name="accv_ps", bufs=2, space="PSUM") as vps:
        for lb in range(NLB):
            xt = [xtstage.tile([128, LB], F32R, tag=f"xt{dc}",
                               name=f"xt{dc}") for dc in range(NDC)]
            rows = []          # rows[lt][half]: [128, DIM//2] tiles
            HD = DIM // 2
            for lt in range(nlt):
                l0 = lb * LB + lt * 128
                halves = []
                for hf in range(2):
                    xr = xstage.tile([128, HD], F32, tag=f"xrow{lt}{hf}",
                                     name=f"xrow{lt}{hf}")
                    nc.sync.dma_start(
                        xr[:], x_dram[l0:l0 + 128, hf * HD:(hf + 1) * HD])
                    halves.append(xr)
                if mask_dram is not None:
                    mt = mstage.tile([128, 1], F32, tag="mt", name="mt")
                    nc.sync.dma_start(mt[:], mask_dram[l0:l0 + 128, :])
                    masked = []
                    for hf in range(2):
                        xm = xstage.tile([128, HD], F32R,
                                         tag=f"xrm{lt}{hf}",
                                         name=f"xrm{lt}{hf}")
                        nc.vector.tensor_scalar_mul(
                            xm[:], halves[hf][:], mt[:])
                        masked.append(xm)
                    halves = masked
                rows.append(halves)
            rounded = mask_dram is not None
            tdt = F32R if rounded else F32
            for dc in range(NDC):
                hf, dco = divmod(dc, NDC // 2)
                pt = tps.tile([128, LB], tdt, tag="tp", name="tp")
                for lt in range(nlt):
                    src = rows[lt][hf][:, dco * 128:(dco + 1) * 128]
                    if rounded:
                        nc.tensor.transpose(pt[:, lt * 128:(lt + 1) * 128],
                                            src, tc.ident_r[:])
                    else:
                        nc.tensor.transpose(pt[:, lt * 128:(lt + 1) * 128],
                                            src, tc.ident[:])
                nc.vector.tensor_copy(xt[dc][:], pt[:])
            # accumulation in waves; acc tags have bufs=2 so the PSUM
            # drain copy of one wave overlaps the next wave's matmuls
            for wi, (wt, kind, dst) in enumerate(wts):
                if kind == "T":
                    for wave in range(2):
                        accs = [aps.tile([128, LB], F32, tag=f"accT{j}",
                                         name=f"accT{j}") for j in range(2)]
                        for dc in range(NDC):
                            for j in range(2):
                                jt = wave * 2 + j
                                nc.tensor.matmul(
                                    accs[j][:],
                                    wt[dc][:, jt * 128:(jt + 1) * 128],
                                    xt[dc][:],
                                    start=(dc == 0), stop=(dc == NDC - 1))
                        for j in range(2):
                            jt = wave * 2 + j
                            nc.vector.tensor_copy(
                                dst[jt][:, lb * LB:(lb + 1) * LB], accs[j][:])
                else:
                    for lt in range(nlt):
                        acc = vps.tile([128, JB], F32, tag="accV",
                                       name="accV")
                        for dc in range(NDC):
                            nc.tensor.matmul(
                                acc[:],
                                xt[dc][:, lt * 128:(lt + 1) * 128],
                                wt[dc][:],
                                start=(dc == 0), stop=(dc == NDC - 1))
                        kt = lb * nlt + lt
                        for h in range(NH):
                            nc.vector.tensor_copy(
                                dst[kt][:, h * 129: h * 129 + 128],
                                acc[:, h * 128:(h + 1) * 128])


def build_kernel(nc, tc, xq, xv, msk, wq, wk, wv, out):
    import contextlib
    octx = contextlib.ExitStack()
    with octx:
        const_pool = octx.enter_context(tc.tile_pool(name="const", bufs=1))
        tc.ident = const_pool.tile([128, 128], F32)
        masks.make_identity(nc, tc.ident[:])
        ident_r = const_pool.tile([128, 128], F32R, name="ident_r")
        nc.vector.tensor_copy(ident_r[:], tc.ident[:])
        tc.ident_r = ident_r

        # persistent products
        qt_pool = octx.enter_context(tc.tile_pool(name="qt", bufs=1))
        kt_pool = octx.enter_context(tc.tile_pool(name="kt", bufs=1))
        v_pool = octx.enter_context(tc.tile_pool(name="v", bufs=1))
        Q_T = [qt_pool.tile([128, L], F32R, tag=f"q{h}", name=f"qT{h}")
               for h in range(NH)]
        K_T = [kt_pool.tile([128, L], F32R, tag=f"k{h}", name=f"kT{h}")
               for h in range(NH)]
        V = [v_pool.tile([128, NH * 129], BF16, tag=f"v{t}", name=f"vS{t}")
             for t in range(NKT)]
        for t in range(NKT):
            nc.vector.memset(V[t][:], 1.0)

        # ---- phase Q ----
        with contextlib.ExitStack() as qctx:
            wqt = transpose_w(nc, tc, qctx, wq, "wq")
            project(nc, tc, qctx, xq, [(wqt, "T", Q_T)], mask_dram=msk)

        # ---- phase K/V ----
        with contextlib.ExitStack() as kvctx:
            wkt = transpose_w(nc, tc, kvctx, wk, "wk")
            wvt = transpose_w(nc, tc, kvctx, wv, "wv")
            project(nc, tc, kvctx, xv, [(wkt, "T", K_T), (wvt, "V", V)])

        # ---- attention ----
        with tc.tile_pool(name="et", bufs=2) as et_pool, \
             tc.tile_pool(name="s_ps", bufs=2, space="PSUM") as sps, \
             tc.tile_pool(name="a_ps", bufs=3, space="PSUM") as aps, \
             tc.tile_pool(name="o_sb", bufs=8) as osb, \
             tc.tile_pool(name="r_sb", bufs=4) as rsb:
            for qb in range(L // 512):
                ots = [osb.tile([128, JB], F32, tag=f"ot{qs}",
                                name=f"ot{qs}") for qs in range(4)]
                for h in range(NH):
                    et = [et_pool.tile([128, 1024], BF16, tag=f"et{k2}",
                                       name=f"et{k2}")
                          for k2 in range(NKT // 2)]
                    for k2 in range(NKT // 2):
                        s = sps.tile([128, 1024], F32, tag="s", name="s")
                        for i in range(2):
                            nc.tensor.matmul(
                                s[:, i * 512:(i + 1) * 512],
                                K_T[h][:, (2 * k2 + i) * 128:
                                        (2 * k2 + i + 1) * 128],
                                Q_T[h][:, qb * 512:(qb + 1) * 512],
                                start=True, stop=True)
                        nc.scalar.activation(
                            et[k2][:], s[:],
                            mybir.ActivationFunctionType.Exp,
                            scale=float(SCALE))
                    for qs in range(4):
                        a = aps.tile([128, 129], F32, tag="a", name="a")
                        for kc in range(NKT):
                            nc.tensor.matmul(
                                a[:],
                                et[kc // 2][:, (kc % 2) * 512
                                            + qs * 128:(kc % 2) * 512
                                            + (qs + 1) * 128],
                                V[kc][:, h * 129:(h + 1) * 129],
                                start=(kc == 0), stop=(kc == NKT - 1))
                        rec = rsb.tile([128, 1], F32, tag="rec", name="rec")
                        nc.vector.reciprocal(rec[:], a[:, 128:129])
                        nc.vector.tensor_scalar_mul(
                            ots[qs][:, h * 128:(h + 1) * 128],
                            a[:, 0:128], rec[:])
                for qs in range(4):
                    q0 = qb * 512 + qs * 128
                    nc.sync.dma_start(out[q0:q0 + 128, :], ots[qs][:])


_NC_CACHE = None


def _get_nc():
    global _NC_CACHE
    if _NC_CACHE is None:
        _NC_CACHE = build_nc()
    return _NC_CACHE


def make_in_maps(query_tensor, value_tensor, attention_mask, Wq, Wk, Wv):
    in_maps = []
    for c in range(N_CORES):
        b, g = divmod(c, 4)
        j0 = g * JB
        in_maps.append({
            "xq": np.ascontiguousarray(query_tensor[b], dtype=np.float32),
            "xv": np.ascontiguousarray(value_tensor[b], dtype=np.float32),
            "msk": np.ascontiguousarray(
                attention_mask[b].reshape(L, 1), dtype=np.float32),
            "wq": np.ascontiguousarray(Wq[j0:j0 + JB], dtype=np.float32),
            "wk": np.ascontiguousarray(Wk[j0:j0 + JB], dtype=np.float32),
            "wv": np.ascontiguousarray(Wv[j0:j0 + JB], dtype=np.float32),
        })
    return in_maps


def assemble(results):
    out = np.empty((B, L, DIM), dtype=np.float32)
    for c in range(N_CORES):
        b, g = divmod(c, 4)
        out[b, :, g * JB:(g + 1) * JB] = results[c]["out"]
    return out


def kernel(query_tensor, value_tensor, attention_mask, Wq, Wk, Wv):
    nc = _get_nc()
    in_maps = make_in_maps(np.asarray(query_tensor), np.asarray(value_tensor),
                           np.asarray(attention_mask), np.asarray(Wq),
                           np.asarray(Wk), np.asarray(Wv))
    res = run_bass_kernel_spmd(nc, in_maps, core_ids=list(range(N_CORES)))
    return assemble(res.results)
